# revision 7
# baseline (speedup 1.0000x reference)
"""Trainium2 Bass kernel for nn_AttGRU (B=16, S=64, N=2048, E=256) on 8 NeuronCores.

Math restructuring (validated in numpy against the reference):
  - scores[b,i,j] = Q_i.K_j with Q = Wq x + bq, K = Wk x + bk is rewritten as
    sT[j,i] = s[i,j] = xh_j^T M xh_i with xh = [x; 1] (65-vector) and
    M = [[G^T, u],[v^T, c]], G = Wq^T Wk, u = Wq^T bk, v = Wk^T bq, c = bq.bk.
    M is computed on the host (65x65), so the whole QK^T contraction is K=65.
  - softmax over i (dim=1) has per-j denominator D[j] = sum_i exp(masked s);
    |s| << 88 so raw exp is fp32-safe; masked entries get -1e30 added inside
    PSUM (via identity-matmuls with the mask as stationary operand) and
    underflow to exactly 0 after exp.
  - agg_t[b,i] = sum_j E[b,i,j]/D[b,j] x[b,t,j]; precomputed for all t as
    AGG[t,i] = sum_j (xT[j,t] * Dinv[j]) E[j,i] (one matmul per (b, j-chunk)).
  - GRU: 64 sequential steps; gate pre-activations via stationary hT chunks
    (M=16) streaming W^T (three gates concatenated per core).

Sharding: attention/gate output dim (i) sharded 8 ways. Each core holds
W^T[:, i-slice] (6 MB) SBUF-resident, computes scores/E/AGG only for its
i-slice (all 16 batches), and the per-step h slice [16, 256]. Cross-core:
one AllReduce per batch for D (pipelined), one 16 KB AllGather per GRU step.
"""

import sys

for _p in ("/opt/trn_rl_repo", "/root/.axon_site/_ro/trn_rl_repo"):
    if _p not in sys.path:
        sys.path.append(_p)

import numpy as np
from contextlib import ExitStack

import concourse.bacc as bacc
import concourse.tile as tile
import concourse.mybir as mybir
from concourse.bass_utils import run_bass_kernel_spmd

B, S, N, E = 16, 64, 2048, 256
NC = 8            # cores
ISL = N // NC     # 256 i per core
JT = N // 128     # 16 j-tiles
SA = S + 1        # 65 augmented contraction dim
G3 = 3 * ISL      # 768 gate-concat output per core
FP32 = mybir.dt.float32
AF = mybir.ActivationFunctionType
NEG = np.float32(-1e30)


# ------------------------------------------------------------------ host prep
def _host_prep(x, adj, Wq, bq, Wk, bk, Whr, bhr, Whz, bhz, Whn, bhn, Wo, bo):
    f64 = np.float64
    x = np.asarray(x, np.float32)

    G = np.asarray(Wq, f64).T @ np.asarray(Wk, f64)
    u = np.asarray(Wq, f64).T @ np.asarray(bk, f64)
    v = np.asarray(Wk, f64).T @ np.asarray(bq, f64)
    c = np.asarray(bq, f64) @ np.asarray(bk, f64)
    # out[j,i] = s[i,j] = xh_j^T M xh_i, M = [[G^T, v],[u^T, c]] (u pairs x_i, v pairs x_j)
    M = np.block([[G.T, v[:, None]], [u[None, :], np.array([[c]])]]).astype(np.float32)
    MT = np.ascontiguousarray(M.T)  # lhsT for H = M @ Xh_slice

    ones_row = np.ones((B, 1, N), np.float32)
    Xh = np.ascontiguousarray(np.concatenate([x, ones_row], axis=1))  # [B, 65, N]

    xT = np.transpose(x, (0, 2, 1))  # [B, N, S]
    xt_tiled = np.ascontiguousarray(
        xT.reshape(B, JT, 128, S).transpose(0, 2, 1, 3).reshape(B, 128, JT * S)
    )

    maskneg = np.where(np.asarray(adj) > 0, np.float32(0), NEG).astype(np.float32)
    I256 = np.eye(256, dtype=np.float32)
    I_tiled = np.ascontiguousarray(
        I256.reshape(2, 128, 256).transpose(1, 0, 2).reshape(128, 512)
    )

    Whs = [np.asarray(Whr, np.float32), np.asarray(Whz, np.float32), np.asarray(Whn, np.float32)]
    ball = np.concatenate([np.asarray(bhr), np.asarray(bhz), np.asarray(bhn)]).astype(np.float32)

    Wo_full = np.asarray(Wo, np.float32).reshape(N)
    Wo_tiled = np.ascontiguousarray(Wo_full.reshape(JT, 128).T)  # [128, 16]
    bo_val = np.asarray(bo, np.float32).reshape(1, 1)

    in_maps = []
    for cid in range(NC):
        isl = slice(cid * ISL, (cid + 1) * ISL)
        Wsl = np.concatenate([Wg.T[:, isl] for Wg in Whs], axis=1)  # [2048, 768]
        W_tiled = np.ascontiguousarray(
            Wsl.reshape(JT, 128, G3).transpose(1, 0, 2).reshape(128, JT * G3)
        )
        mask_tiled = np.ascontiguousarray(
            maskneg[isl, :].reshape(2, 128, N).transpose(1, 0, 2).reshape(128, 2 * N)
        )
        xhs = np.ascontiguousarray(Xh[:, :, isl])  # [B, 65, 256]
        b3 = np.concatenate(
            [ball[isl], ball[N + cid * ISL : N + (cid + 1) * ISL], np.zeros(ISL, np.float32)]
        )
        b3_rep = np.ascontiguousarray(np.broadcast_to(b3, (S, G3)))
        bn_rep = np.ascontiguousarray(
            np.broadcast_to(ball[2 * N + cid * ISL : 2 * N + (cid + 1) * ISL], (B, ISL))
        )
        in_maps.append(
            dict(
                xh=Xh, xhs=xhs, xt=xt_tiled, mt=MT,
                mask=mask_tiled, ident=I_tiled, w=W_tiled,
                b3=b3_rep, bn=bn_rep, wo=Wo_tiled, bo=bo_val,
            )
        )
    return in_maps


# ------------------------------------------------------------------ kernel IR
def _kernel_body(tc, d):
    nc = tc.nc
    RG = [list(range(NC))]

    with ExitStack() as ctx:
        const_pool = ctx.enter_context(tc.tile_pool(name="const", bufs=1))
        dram = ctx.enter_context(tc.tile_pool(name="dramscratch", bufs=1, space="DRAM"))

        mask_sb = const_pool.tile([128, 2 * N], FP32)
        nc.sync.dma_start(mask_sb[:], d["mask"])
        id_sb = const_pool.tile([128, 512], FP32)
        nc.sync.dma_start(id_sb[:], d["ident"])
        mt_sb = const_pool.tile([SA, SA], FP32)
        nc.sync.dma_start(mt_sb[:], d["mt"])
        b3_sb = const_pool.tile([S, G3], FP32)
        nc.sync.dma_start(b3_sb[:], d["b3"])
        bn_sb = const_pool.tile([B, ISL], FP32)
        nc.sync.dma_start(bn_sb[:], d["bn"])
        wo_sb = const_pool.tile([128, JT], FP32)
        nc.sync.dma_start(wo_sb[:], d["wo"])
        bo_sb = const_pool.tile([1, 1], FP32)
        nc.sync.dma_start(bo_sb[:], d["bo"])
        w_sb = const_pool.tile([128, JT * G3], FP32)
        nc.sync.dma_start(w_sb[:], d["w"])

        agg3_dram = dram.tile([B, S, G3], FP32)

        # ========================= phase A/B =========================
        with ExitStack() as actx:
            xh_pool = actx.enter_context(tc.tile_pool(name="xhp", bufs=2))
            small_pool = actx.enter_context(tc.tile_pool(name="smallp", bufs=2))
            e_pool = actx.enter_context(tc.tile_pool(name="ep", bufs=2))
            s_psum = actx.enter_context(tc.tile_pool(name="spsum", bufs=3, space="PSUM"))
            h_psum = actx.enter_context(tc.tile_pool(name="hpsum", bufs=2, space="PSUM"))
            g_psum = actx.enter_context(tc.tile_pool(name="gpsum", bufs=2, space="PSUM"))
            ar_dram = actx.enter_context(tc.tile_pool(name="ardram", bufs=2, space="DRAM"))

            for b in range(B):
                xh_sb = xh_pool.tile([SA, N], FP32, tag="xh")
                nc.sync.dma_start(xh_sb[:], d["xh"][b])
                xhs_sb = small_pool.tile([SA, ISL], FP32, tag="xhs")
                nc.sync.dma_start(xhs_sb[:], d["xhs"][b])
                xt_sb = small_pool.tile([128, JT * S], FP32, tag="xt")
                nc.sync.dma_start(xt_sb[:], d["xt"][b])

                # H = M @ Xh[:, islice]  -> [65, 256]
                h_ps = h_psum.tile([SA, ISL], FP32, tag="hps")
                nc.tensor.matmul(h_ps[:], mt_sb[:], xhs_sb[:], start=True, stop=True)
                h_sb = small_pool.tile([SA, ISL], FP32, tag="hsb")
                nc.scalar.copy(h_sb[:], h_ps[:])

                # E tiles: e_sb[p, jt*256 + i] = exp(s[i, jt*128+p] + maskneg)
                e_sb = e_pool.tile([128, JT * ISL], FP32, tag="esb")
                for jt in range(JT):
                    s_ps = s_psum.tile([128, ISL], FP32, tag="sps")
                    nc.tensor.matmul(
                        s_ps[:], mask_sb[:, jt * 128 : (jt + 1) * 128],
                        id_sb[:, 0:ISL], start=True, stop=False,
                    )
                    nc.tensor.matmul(
                        s_ps[:], mask_sb[:, N + jt * 128 : N + (jt + 1) * 128],
                        id_sb[:, ISL : 2 * ISL], start=False, stop=False,
                    )
                    nc.tensor.matmul(
                        s_ps[:], xh_sb[:, jt * 128 : (jt + 1) * 128],
                        h_sb[:], start=False, stop=True,
                    )
                    nc.scalar.activation(
                        e_sb[:, jt * ISL : (jt + 1) * ISL], s_ps[:], AF.Exp
                    )

                # D partial = sum_i E  (one 3D reduce)
                d_sb = small_pool.tile([128, JT], FP32, tag="dsb")
                nc.vector.tensor_reduce(
                    d_sb[:], e_sb[:].rearrange("p (j i) -> p j i", i=ISL),
                    axis=mybir.AxisListType.X, op=mybir.AluOpType.add,
                )
                ar_in = ar_dram.tile([128, JT], FP32, tag="arin")
                nc.sync.dma_start(ar_in[:], d_sb[:])
                ar_out = ar_dram.tile([128, JT], FP32, tag="arout")
                nc.gpsimd.collective_compute(
                    "AllReduce", mybir.AluOpType.add, replica_groups=RG,
                    ins=[ar_in.opt()], outs=[ar_out.opt()],
                )
                df_sb = small_pool.tile([128, JT], FP32, tag="dfsb")
                nc.sync.dma_start(df_sb[:], ar_out[:])
                dinv_sb = small_pool.tile([128, JT], FP32, tag="dinv")
                nc.vector.reciprocal(dinv_sb[:], df_sb[:])

                # AGG[t, i] = sum_j (xT[j,t] * Dinv[j]) E[j, i]
                xd_sb = small_pool.tile([128, JT * S], FP32, tag="xdsb")
                agg_ps = g_psum.tile([S, ISL], FP32, tag="aggps")
                for jt in range(JT):
                    nc.vector.tensor_scalar_mul(
                        xd_sb[:, jt * S : (jt + 1) * S],
                        xt_sb[:, jt * S : (jt + 1) * S],
                        dinv_sb[:, jt : jt + 1],
                    )
                    nc.tensor.matmul(
                        agg_ps[:], xd_sb[:, jt * S : (jt + 1) * S],
                        e_sb[:, jt * ISL : (jt + 1) * ISL],
                        start=(jt == 0), stop=(jt == JT - 1),
                    )

                # agg3 = [agg + bhr | agg + bhz | agg] -> DRAM[b]
                agg_sb = small_pool.tile([S, G3], FP32, tag="aggsb")
                nc.vector.tensor_add(agg_sb[:, 0:ISL], agg_ps[:], b3_sb[:, 0:ISL])
                nc.vector.tensor_add(
                    agg_sb[:, ISL : 2 * ISL], agg_ps[:], b3_sb[:, ISL : 2 * ISL]
                )
                nc.scalar.copy(agg_sb[:, 2 * ISL : G3], agg_ps[:])
                nc.sync.dma_start(agg3_dram[b], agg_sb[:])

        # ========================= phase C =========================
        with ExitStack() as cctx:
            ht_pool = cctx.enter_context(tc.tile_pool(name="htp", bufs=2))
            gate_pool = cctx.enter_context(tc.tile_pool(name="gatep", bufs=2))
            aggt_pool = cctx.enter_context(tc.tile_pool(name="aggtp", bufs=3))
            c_psum = cctx.enter_context(tc.tile_pool(name="cpsum", bufs=2, space="PSUM"))
            t_psum = cctx.enter_context(tc.tile_pool(name="tpsum", bufs=2, space="PSUM"))
            ag_dram = cctx.enter_context(tc.tile_pool(name="agdram", bufs=2, space="DRAM"))

            ht_sb = ht_pool.tile([128, JT * B], FP32, tag="ht")
            nc.vector.memset(ht_sb[:], 0.0)
            h_sb = gate_pool.tile([B, ISL], FP32, tag="hsl")
            nc.vector.memset(h_sb[:], 0.0)

            aggt_sb = aggt_pool.tile([B, G3], FP32, tag="aggt")
            nc.sync.dma_start(aggt_sb[:], agg3_dram[:, 0, :])

            for t in range(S):
                pre_ps = c_psum.tile([B, G3], FP32, tag="preps")
                for jc in range(JT):
                    lhsT = ht_sb[:, jc * B : (jc + 1) * B]
                    nc.tensor.matmul(
                        pre_ps[:, 0:512], lhsT, w_sb[:, jc * G3 : jc * G3 + 512],
                        start=(jc == 0), stop=(jc == JT - 1),
                    )
                    nc.tensor.matmul(
                        pre_ps[:, 512:G3], lhsT, w_sb[:, jc * G3 + 512 : (jc + 1) * G3],
                        start=(jc == 0), stop=(jc == JT - 1),
                    )

                # prefetch next agg (off critical path)
                if t + 1 < S:
                    aggt_next = aggt_pool.tile([B, G3], FP32, tag="aggt")
                    nc.sync.dma_start(aggt_next[:], agg3_dram[:, t + 1, :])

                # gates
                rzin = gate_pool.tile([B, 2 * ISL], FP32, tag="rzin")
                nc.vector.tensor_add(rzin[:], pre_ps[:, 0 : 2 * ISL], aggt_sb[:, 0 : 2 * ISL])
                rz = gate_pool.tile([B, 2 * ISL], FP32, tag="rz")
                nc.scalar.activation(rz[:], rzin[:], AF.Sigmoid)
                nt1 = gate_pool.tile([B, ISL], FP32, tag="nt1")
                nc.vector.tensor_add(nt1[:], pre_ps[:, 2 * ISL : G3], bn_sb[:])
                nt2 = gate_pool.tile([B, ISL], FP32, tag="nt2")
                nc.vector.tensor_mul(nt2[:], nt1[:], rz[:, 0:ISL])
                nin = gate_pool.tile([B, ISL], FP32, tag="nin")
                nc.vector.tensor_add(nin[:], nt2[:], aggt_sb[:, 2 * ISL : G3])
                ng = gate_pool.tile([B, ISL], FP32, tag="ng")
                nc.scalar.activation(ng[:], nin[:], AF.Tanh)
                hmn = gate_pool.tile([B, ISL], FP32, tag="hmn")
                nc.vector.tensor_sub(hmn[:], h_sb[:], ng[:])
                zh = gate_pool.tile([B, ISL], FP32, tag="zh")
                nc.vector.tensor_mul(zh[:], rz[:, ISL : 2 * ISL], hmn[:])
                h_new = gate_pool.tile([B, ISL], FP32, tag="hsl")
                nc.vector.tensor_add(h_new[:], zh[:], ng[:])
                h_sb = h_new
                aggt_sb = aggt_next if t + 1 < S else aggt_sb

                # transpose h slice -> [128, 16] x2, AllGather, reload hT
                tp_sb = gate_pool.tile([128, 2 * B], FP32, tag="tpsb")
                for cch in range(2):
                    tp_ps = t_psum.tile([128, B], FP32, tag="tpps")
                    nc.tensor.transpose(
                        tp_ps[:], h_new[:, cch * 128 : (cch + 1) * 128], id_sb[0:B, 0:B]
                    )
                    nc.scalar.copy(tp_sb[:, cch * B : (cch + 1) * B], tp_ps[:])
                ag_in = ag_dram.tile([2 * 128, B], FP32, tag="agin")
                nc.sync.dma_start(
                    ag_in[:].rearrange("(c p) b -> p c b", p=128),
                    tp_sb[:].rearrange("p (c b) -> p c b", c=2),
                )
                ag_out = ag_dram.tile([N, B], FP32, tag="agout")
                nc.gpsimd.collective_compute(
                    "AllGather", mybir.AluOpType.bypass, replica_groups=RG,
                    ins=[ag_in.opt()], outs=[ag_out.opt()],
                )
                ht_sb = ht_pool.tile([128, JT * B], FP32, tag="ht")
                nc.sync.dma_start(
                    ht_sb[:].rearrange("p (c b) -> p c b", c=JT),
                    ag_out[:].rearrange("(c p) b -> p c b", p=128),
                )

            # output head: out[b] = sum_j h[b, j] Wo[j] + bo  (full h from last AG)
            out_ps = t_psum.tile([1, B], FP32, tag="outps")
            for jc in range(JT):
                nc.tensor.matmul(
                    out_ps[:], wo_sb[:, jc : jc + 1], ht_sb[:, jc * B : (jc + 1) * B],
                    start=(jc == 0), stop=(jc == JT - 1),
                )
            out_sb = gate_pool.tile([1, B], FP32, tag="outsb")
            nc.vector.tensor_scalar_add(out_sb[:], out_ps[:], bo_sb[0:1, 0:1])
            nc.sync.dma_start(d["out"], out_sb[:])


def _build():
    nc = bacc.Bacc("TRN2", target_bir_lowering=False, debug=False, num_devices=NC)
    d = dict(
        xh=nc.dram_tensor("xh", [B, SA, N], FP32, kind="ExternalInput").ap(),
        xhs=nc.dram_tensor("xhs", [B, SA, ISL], FP32, kind="ExternalInput").ap(),
        xt=nc.dram_tensor("xt", [B, 128, JT * S], FP32, kind="ExternalInput").ap(),
        mt=nc.dram_tensor("mt", [SA, SA], FP32, kind="ExternalInput").ap(),
        mask=nc.dram_tensor("mask", [128, 2 * N], FP32, kind="ExternalInput").ap(),
        ident=nc.dram_tensor("ident", [128, 512], FP32, kind="ExternalInput").ap(),
        w=nc.dram_tensor("w", [128, JT * G3], FP32, kind="ExternalInput").ap(),
        b3=nc.dram_tensor("b3", [S, G3], FP32, kind="ExternalInput").ap(),
        bn=nc.dram_tensor("bn", [B, ISL], FP32, kind="ExternalInput").ap(),
        wo=nc.dram_tensor("wo", [128, JT], FP32, kind="ExternalInput").ap(),
        bo=nc.dram_tensor("bo", [1, 1], FP32, kind="ExternalInput").ap(),
        out=nc.dram_tensor("out", [1, B], FP32, kind="ExternalOutput").ap(),
    )
    with tile.TileContext(nc) as tc:
        _kernel_body(tc, d)
    nc.compile()  # bacc register allocation / DCE / fusion
    return nc


def run_with_results(inputs, trace=False, **kw):
    in_maps = _host_prep(**inputs)
    nc = _build()
    res = run_bass_kernel_spmd(
        nc, in_maps, core_ids=list(range(NC)), trace=trace, **kw
    )
    out = np.asarray(res.results[0]["out"], np.float32).reshape(B)
    return out, res


def kernel(**inputs) -> np.ndarray:
    out, _ = run_with_results(inputs)
    return out


if __name__ == "__main__":
    import reference

    inputs = {k: np.asarray(v) for k, v in reference.setup_inputs().items()}
    out = kernel(**inputs)
    print("kernel out:", out)
